# revision 49
# baseline (speedup 1.0000x reference)
"""CovaBlock Trainium2 kernel (nn_CovaBlock_8589934592087).

reference:
  support (16,5,256,21,21) -> per-class covariance cov (16,256,256)
  query (64,256,21,21) -> l2-normalize over C per location ->
  sim[b,k,l] = qn[b,:,l]^T cov[k] qn[b,:,l]  -> out (64, 16*441)

Distribution over 8 NeuronCores:
  stage 1: shard K (2 classes/core) -> each core computes 2 covariance
           matrices in bf16; host gathers to (16,256,256) bf16.
  stage 2: data-parallel over B (8 queries/core), cov replicated.

Host-side work (exact, cheap): fp8/bf16 conversion of q/support,
support pre-transposed to (K, Shot*L, C) so stage 1 needs no on-device
transposes, the covariance mean-correction and 1/(N-1) scale, and the
l2 normalization folded into a final scale by 1/(||q||+1e-8)^2 applied
to the raw quadratic form the device computes.

Stage-2 per-core algorithm (all matmuls fp8-e4m3, f32 PSUM accum):
  M1[b,k] = cov[k]^T @ q[b]                    (C x 441, tensor engine)
  PSUM->SBUF fp8 copies of M1 split ~2:1 over Act/DVE (GPSIMD cannot
       read PSUM on HW; free-axis reduce is DVE-only)
  S[l',(k,c)] = sum_d q[b][d,l'] M1[b][d,k,c]  (col-tiled 32-wide l'
       subtiles so only diagonal 32x32 blocks of the Gram are computed)
  sim[l,k] = sum_c S[l,(k,c)] * blockdiag_mask (DVE mult+reduce)
  out[b,(g,k),l128] via one 128x64 transpose -> 512B-line DMA

fp8 DoubleRow (2x PE rate) is implemented behind COVA_NODR=0 but is
disabled: at full kernel scale any NEFF mixing DoubleRow matmuls with
tile_position'd matmuls dies with NRT_EXEC_UNIT_UNRECOVERABLE, even
phase-separated or drain-fenced (small-scale probes pass).
"""
import sys

for _p in ("/opt/trn_rl_repo",):
    if _p not in sys.path:
        sys.path.insert(0, _p)

import numpy as np
import ml_dtypes

import concourse.bass as bass
import concourse.mybir as mybir
import concourse.tile as tile
from concourse import bass_utils

F32 = mybir.dt.float32
BF16 = mybir.dt.bfloat16
F8 = mybir.dt.float8e4
BFNP = ml_dtypes.bfloat16
F8NP = ml_dtypes.float8_e4m3

USE_FP8_MM1 = True
import os as _os
STRIP = int(_os.environ.get("COVA_STRIP", "0"))
NODR = int(_os.environ.get("COVA_NODR", "1"))

# problem shapes (hardcoded per spec)
B, C, H, W = 64, 256, 21, 21
K, SHOT = 16, 5
L = H * W            # 441
LO = 512             # output L stride (4*128 for clean transpose/DMA)
N = SHOT * L         # 2205 support locations per class
NT = (N + 127) // 128  # 18 (last tile 29 rows)
NCORES = 8
B_LOC = B // NCORES  # 8 queries per core
K_LOC = K // NCORES  # 2 classes per core


# ---------------------------------------------------------------- waitfix
def _split_waits(nc):
    """This walrus build accepts at most ONE sync-wait command per
    instruction; hoist excess waits onto preceding NoOps (same engine)."""
    n_split = 0
    for fn in nc.m.functions:
        for blk in fn.blocks:
            new = []
            dirty = False
            for inst in blk.instructions:
                si = inst.sync_info
                waits = list(si.on_wait) if si is not None and si.on_wait else []
                if len(waits) > 1:
                    keep = waits[-1:]
                    for j, w in enumerate(waits[:-1]):
                        nop = mybir.InstNoOp(
                            name=f"{inst.name}-wsplit{j}", ins=[], outs=[]
                        )
                        nop.engine = inst.engine
                        nop.sync_info = mybir.SyncInfo(on_wait=[w], on_update=[])
                        new.append(nop)
                    inst.sync_info = mybir.SyncInfo(
                        on_wait=keep,
                        on_update=list(si.on_update) if si.on_update else [],
                    )
                    n_split += 1
                    dirty = True
                new.append(inst)
            if dirty:
                blk.instructions = new
    return n_split


# ---------------------------------------------------------------- stage 1
def build_stage1(split=True):
    """Per core: pre-transposed support (K_LOC, N, C) bf16
    -> raw gram cov_raw = XT^T XT  (K_LOC,C,C) f32 straight from PSUM.

    The 1/(N-1) scale and the mean correction - m1 m1^T / (N (N-1)) are
    applied on the host (m1 computed there from the f32 support), so the
    device kernel is a pure accumulation with a minimal drain.
    """
    nc = bass.Bass("TRN2", target_bir_lowering=False, debug=False)
    supt = nc.dram_tensor("support_t", [K_LOC, N, C], BF16,
                          kind="ExternalInput").ap()
    cov_out = nc.dram_tensor("cov_out", [K_LOC, C, C], F32,
                             kind="ExternalOutput").ap()

    with tile.TileContext(nc) as tc:
        with (
            tc.tile_pool(name="xtpool", bufs=2) as xtpool,
            tc.tile_pool(name="covsb", bufs=2) as covsb,
            tc.tile_pool(name="cov_ps", bufs=1, space="PSUM") as cov_ps,
        ):
            # load XT for both classes, chunked (small first chunk so the
            # matmuls start early); one DMA queue per class so the loads
            # run in parallel
            xts = []
            for k in range(K_LOC):
                dq = nc.sync if k == 0 else nc.scalar
                xt = xtpool.tile([128, NT, C], BF16, name=f"xt{k}")
                for c0, c1 in ((0, 2), (2, 7), (7, 12), (12, 17)):
                    dq.dma_start(
                        xt[:, c0:c1, :],
                        supt[k, c0 * 128:c1 * 128, :]
                        .rearrange("(t p) c -> p t c", p=128),
                    )
                dq.dma_start(xt[0:29, 17, :], supt[k, 17 * 128:N, :])
                xts.append(xt)

            # classes run sequentially: class 0 is DMA-paced while class
            # 1's load streams in parallel, and class 0's output DMA
            # overlaps class 1's matmuls
            for k in range(K_LOC):
                xt = xts[k]
                cps = [cov_ps.tile([128, C], F32, name=f"cp{k}_{ct2}")
                       for ct2 in range(2)]
                for nt in range(NT):
                    w = min(128, N - nt * 128)
                    for ct2 in range(2):
                        nc.tensor.matmul(
                            cps[ct2][:],
                            xt[0:w, nt, ct2 * 128:(ct2 + 1) * 128],
                            xt[0:w, nt, :],
                            start=(nt == 0), stop=(nt == NT - 1),
                        )
                for ct2 in range(2):
                    csb = covsb.tile([128, C], F32, name=f"csb{k}_{ct2}")
                    # GPSIMD cannot access PSUM on HW: Act/DVE only
                    if ct2 == 0:
                        nc.vector.tensor_copy(csb[:], cps[ct2][:])
                    else:
                        nc.scalar.copy(csb[:], cps[ct2][:])
                    dq = nc.sync if ct2 == 0 else nc.scalar
                    dq.dma_start(
                        cov_out[k, ct2 * 128:(ct2 + 1) * 128, :], csb[:]
                    )
    if split:
        _split_waits(nc)
    return nc


# ---------------------------------------------------------------- stage 2
def build_stage2(split=True):
    """Per core: q shard (B_LOC,C,L) fp8 + cov (K,C,C) fp8
    -> raw quadratic form out (B_LOC, K, LO) f32 (host applies 1/norm^2).

    Phase A: M1[b,k] = cov_k^T q_b via fp8 matmuls (DoubleRow behind
    COVA_NODR=0 -- see module docstring), PSUM->SBUF fp8 copies on
    Act/DVE.  Phase B: 32-wide diagonal Gram blocks (fp8,
    tile_position-packed), mask+reduce on DVE, one 128x64 transpose,
    512B-line output DMA.
    """
    nc = bass.Bass("TRN2", target_bir_lowering=False, debug=False)
    cov_in = nc.dram_tensor("cov", [K, C, C], BF16, kind="ExternalInput").ap()
    qm_in = nc.dram_tensor("qm", [B_LOC, C, L], BF16, kind="ExternalInput").ap()
    mask_in = nc.dram_tensor("mask", [128, K, 32], F32,
                             kind="ExternalInput").ap()
    idf_in = nc.dram_tensor("identity", [128, 128], F32,
                            kind="ExternalInput").ap()
    out = nc.dram_tensor("out", [B_LOC, K, LO], F32, kind="ExternalOutput").ap()

    with tile.TileContext(nc) as tc:
        with (
            tc.tile_pool(name="singles", bufs=1) as singles,
            tc.tile_pool(name="m1pool", bufs=1) as m1pool,
            tc.tile_pool(name="prodp", bufs=3) as prodp,
            tc.tile_pool(name="simp", bufs=2) as simp,
            tc.tile_pool(name="outp", bufs=2) as outp,
            tc.tile_pool(name="m1_ps", bufs=4, space="PSUM") as m1_ps,
            tc.tile_pool(name="s_ps", bufs=2, space="PSUM") as s_ps,
            tc.tile_pool(name="o_ps", bufs=2, space="PSUM") as o_ps,
        ):
            qmm = singles.tile([128, 2, B_LOC, L], BF16)
            cov_sb = singles.tile([128, K, 2, C], BF16)
            mask = singles.tile([128, K, 32], F32)
            idf32 = singles.tile([128, 128], F32)

            # one DMA queue, ordered by consumption (descriptor gen is a
            # shared serial resource): cov in 4-class batches interleaved
            # with the early queries
            def dma_qm(b):
                nc.sync.dma_start(
                    qmm[:, :, b, :],
                    qm_in[b].rearrange("(ct p) l -> p ct l", p=128),
                )

            def dma_cov4(k0):
                nc.sync.dma_start(
                    cov_sb[:, k0:k0 + 4, :, :],
                    cov_in[k0:k0 + 4]
                    .rearrange("k (ct p) d -> p k ct d", p=128),
                )

            dma_cov4(0)
            dma_qm(0)
            dma_qm(1)
            dma_cov4(4)
            dma_qm(2)
            dma_qm(3)
            dma_cov4(8)
            dma_cov4(12)
            for b in range(4, B_LOC):
                dma_qm(b)
            nc.gpsimd.dma_start(mask[:], mask_in)
            nc.gpsimd.dma_start(idf32[:], idf_in)

            # M1 PSUM->SBUF copies: GPSIMD cannot access PSUM on HW;
            # DVE also runs the extracts, so copies split ~2:1 Act:DVE
            cri = [0]

            def copy_any(dst, src, i):
                if i == 0:
                    nc.scalar.copy(dst, src)
                else:
                    cri[0] += 1
                    if cri[0] % 3 == 0:
                        nc.scalar.copy(dst, src)
                    else:
                        nc.vector.tensor_copy(dst, src)

            # ---------------- phase B: diag blocks + extract + out -----
            def emit_second_mm(b, m1b, tail=False):
                sim = simp.tile([128, 4, K], F32, name=f"sim_{b}")
                # non-zero partition starts allow at most 32 partitions
                # per access; TR rewrites [32:57)
                for p0 in (32, 64, 96):
                    nc.gpsimd.memset(sim[p0:p0 + 32, 3, :], 0.0)
                for ltg in range(4):
                    npart = 128 if ltg < 3 else 57
                    nj = 4 if ltg < 3 else 2
                    sps = s_ps.tile([128, K, 32], F32, tag="sps")
                    for j in range(nj):
                        lo = ltg * 128 + j * 32
                        w = min(32, L - lo)
                        for dt in range(2):
                            nc.tensor.matmul(
                                sps[32 * j:32 * j + w, :, 0:w],
                                qmm[:, dt, b, lo:lo + w],
                                m1b[:, dt, :, lo:lo + w],
                                start=(dt == 0), stop=(dt == 1),
                                tile_position=(0, 32 * j),
                            )
                    prod = prodp.tile([128, K, 32], F32)
                    # sps is PSUM: GPSIMD may not touch it -> DVE
                    nc.vector.tensor_tensor(
                        prod[0:npart], sps[0:npart], mask[0:npart],
                        op=mybir.AluOpType.mult,
                    )
                    nc.vector.tensor_reduce(
                        sim[0:npart, ltg, :], prod[0:npart],
                        axis=mybir.AxisListType.X, op=mybir.AluOpType.add,
                    )
                if not tail:
                    # one transpose: sim [128l, (g,k)=64] -> [(g,k), 128l]
                    ops_ = o_ps.tile([64, 128], F32)
                    nc.tensor.transpose(
                        ops_[:, :], sim[:].rearrange("p g k -> p (g k)"),
                        idf32[:],
                    )
                    outsb = outp.tile([64, 128], F32)
                    nc.scalar.copy(outsb[:], ops_[:])
                    for g in range(4):
                        nc.sync.dma_start(
                            out[b, :, g * 128:(g + 1) * 128],
                            outsb[g * 16:(g + 1) * 16, :],
                        )
                else:
                    # last query: per-g transpose/copy/DMA pipeline so the
                    # drain is one g-column deep instead of the whole row
                    for g in range(4):
                        ops_ = o_ps.tile([64, 128], F32)
                        nc.tensor.transpose(ops_[0:16, :], sim[:, g, :],
                                            idf32[:])
                        outsb = outp.tile([64, 128], F32)
                        nc.scalar.copy(outsb[0:16, :], ops_[0:16, :])
                        nc.sync.dma_start(
                            out[b, :, g * 128:(g + 1) * 128], outsb[0:16, :]
                        )

            # ---------------- phases: 2 superphases of 4 queries -------
            for sp in range(2):
              bs = [4 * sp + i for i in range(4)]
              m1s = {b: m1pool.tile([128, 2, K, L], BF16,
                                    name=f"m1_{b % 4}") for b in bs}
              for bp in (0, 1):
                bpair = (bs[2 * bp], bs[2 * bp + 1])
                for k in range(K):
                    for dt in range(2):
                        mps = [m1_ps.tile([128, L], F32, tag="m1ps",
                                          name=f"m1ps_{b}") for b in bpair]
                        for i, b in enumerate(bpair):
                            if NODR:
                                for ct in range(2):
                                    nc.tensor.matmul(
                                        mps[i][:],
                                        cov_sb[:, k, ct,
                                               dt * 128:(dt + 1) * 128],
                                        qmm[:, ct, b, :],
                                        start=(ct == 0), stop=(ct == 1),
                                    )
                            else:
                                # moving free caps at 512 = 2 k-tiles x 256
                                for lo, hi in ((0, 221), (221, L)):
                                    nc.tensor.matmul(
                                        mps[i][:, lo:hi],
                                        cov_sb[:, k, :,
                                               dt * 128:(dt + 1) * 128],
                                        qmm[:, :, b, lo:hi],
                                        start=True, stop=True,
                                        perf_mode=(
                                            mybir.MatmulPerfMode.DoubleRow),
                                    )
                        for i, b in enumerate(bpair):
                            copy_any(m1s[b][:, dt, k, :], mps[i][:], i)
              for b in bs:
                  emit_second_mm(b, m1s[b], tail=(b == B_LOC - 1))


    if split:
        _split_waits(nc)
    return nc


# ---------------------------------------------------------------- host
_CACHE = {}


def _get(name):
    if name not in _CACHE:
        _CACHE[name] = build_stage1() if name == "s1" else build_stage2()
    return _CACHE[name]


def _mask_np():
    m = np.zeros((128, K, 32), dtype=np.float32)
    for p in range(128):
        m[p, :, p % 32] = 1.0
    return m


LAST_RESULTS = {}


def kernel(query_features, support_features):
    q = np.asarray(query_features, dtype=np.float32).reshape(B, C, L)
    sup = np.asarray(support_features, dtype=np.float32).reshape(K, SHOT, C, L)

    # exact normalization folded into a final host-side scale
    n2 = np.einsum("bcl,bcl->bl", q, q, dtype=np.float64)
    inv = (1.0 / (np.sqrt(n2) + 1e-8) ** 2).astype(np.float32)

    qm = q.astype(BFNP)
    # pre-transposed support: (K, Shot, C, L) -> (K, Shot*L, C)
    supt = np.ascontiguousarray(
        sup.astype(BFNP).transpose(0, 1, 3, 2)
    ).reshape(K, N, C)
    id_f32 = np.eye(128, dtype=np.float32)
    mask = _mask_np()

    nc1 = _get("s1")
    in1 = [{"support_t": supt[i * K_LOC:(i + 1) * K_LOC]}
           for i in range(NCORES)]
    r1 = bass_utils.run_bass_kernel_spmd(nc1, in1, core_ids=list(range(NCORES)))
    cov_raw = np.concatenate([r["cov_out"] for r in r1.results], axis=0)

    # host-side scale + mean correction:
    # cov = cov_raw/(N-1) - m1 m1^T / (N (N-1))
    m1 = sup.sum(axis=(1, 3))                      # (K, C) in f32
    corr = np.einsum("kc,kd->kcd", m1, m1) / float(N)
    covf = (cov_raw.astype(np.float32) - corr) / (N - 1.0)
    cov = covf.astype(BFNP)

    nc2 = _get("s2")
    in2 = [{"qm": qm[i * B_LOC:(i + 1) * B_LOC], "cov": cov, "mask": mask,
            "identity": id_f32} for i in range(NCORES)]
    r2 = bass_utils.run_bass_kernel_spmd(nc2, in2, core_ids=list(range(NCORES)))
    outv = np.concatenate([r["out"] for r in r2.results], axis=0)

    LAST_RESULTS["s1"] = r1
    LAST_RESULTS["s2"] = r2
    res = outv[:, :, :L] * inv[:, None, :]
    return res.reshape(B, K * L).astype(np.float32)
